# revision 16
# baseline (speedup 1.0000x reference)
"""AdaLoRAWithBase distributed Trainium2 kernel (8 NeuronCores).

Reference computation (B=16, D=2048, ADA=1024, INTER=1024, R=8):
    h   = gelu(ada_emb @ w1 + b1)                  [B, INTER]
    xw  = h @ w2 + b2                              [B, 2*D*R]
    x_a = xw[:, :D*R]  -> [B, D, R]
    x_b = xw[:, D*R:]  -> [B, D, R]
    layer = base + einsum('bdr,bkr->bdk', x_a, x_b)
    out = x + einsum('bc,bco->bo', x, layer)

Key algebra: the B x D x D layer never needs to be materialized:
    out = x + x @ base + einsum('br,bor->bo', t, x_b)
    t[b,r] = sum_d x[b,d] * x_a[b,d,r]

Distribution (NOT the data-parallel hint -- replicating w2 would force
every core to read all 128MB of it; instead w2's columns are sharded so
each core reads 1/8):
  - core i owns d-range/o-range [i*256, (i+1)*256): the matching 2048
    x_a columns of w2 (permuted r-major), 2048 x_b columns (natural
    o-major) and 256 base columns.
  - h-generation is replicated (w1 is only 2MB in bf16) -- measured on
    this runtime, an AllGather costs ~30-50us of barrier+latency, far
    more than the redundant compute.
  - each core computes t_partial over its d-range; one tiny [16,8]
    AllReduce (overlapped with the x_b-half DMA stream) completes t.
  - each core emits out[:, i*256:(i+1)*256]; the host concatenates.

All matmul operands are bf16 (1 cycle/row on the PE vs 4 for fp32, and
half the DMA bytes); PSUM accumulation stays fp32. Weight DMAs alternate
between the sync and scalar HWDGE queues for bandwidth.
"""

import sys

import numpy as np

for _p in ("/opt/trn_rl_repo",):
    if _p not in sys.path:
        sys.path.insert(0, _p)

from concourse import bacc, bass, mybir, tile
from concourse.bass_utils import run_bass_kernel_spmd

B, D, ADA, INTER, RANK = 16, 2048, 1024, 1024, 8
NC = 8
DS = D // NC          # 256: per-core d/o slice
KA = ADA // 128       # 8 k-tiles for the h matmul
KH = INTER // 128     # 8 k-tiles (and m-tiles) for h / xw
KX = D // 128         # 16 k-tiles for the base matmul
ACOLS = DS * RANK     # 2048 x_a columns per core
F32 = mybir.dt.float32
BF16 = mybir.dt.bfloat16
RG = [list(range(NC))]

_CACHED_NC = None


def build_nc(skip_ar=False):
    nc = bacc.Bacc(
        "TRN2",
        target_bir_lowering=False,
        debug=False,
        num_devices=NC,
    )

    ada_d = nc.declare_dram_parameter("ada", [128, KA * B], BF16, isOutput=False)
    w1_d = nc.declare_dram_parameter("w1f", [128, KH * KA * 128], BF16, isOutput=False)
    b1_d = nc.declare_dram_parameter("b1f", [128, KH], F32, isOutput=False)
    w2_d = nc.declare_dram_parameter("w2s", [128, KH * 2 * ACOLS], BF16, isOutput=False)
    b2_d = nc.declare_dram_parameter("b2s", [1, 2 * ACOLS], BF16, isOutput=False)
    xt_d = nc.declare_dram_parameter("xts", [128, KX * B], BF16, isOutput=False)
    xs_d = nc.declare_dram_parameter("xs", [B, DS], F32, isOutput=False)
    base_d = nc.declare_dram_parameter("bases", [128, KX * DS], BF16, isOutput=False)
    ones_d = nc.declare_dram_parameter("ones", [1, B], BF16, isOutput=False)
    ident_d = nc.declare_dram_parameter("ident", [B, B], BF16, isOutput=False)
    xsb_d = nc.declare_dram_parameter("xsb", [B, DS], BF16, isOutput=False)
    dum_d = nc.declare_dram_parameter("warm", [1, 1], F32, isOutput=False)
    out_d = nc.declare_dram_parameter("out", [B, DS], F32, isOutput=True)

    with tile.TileContext(nc) as tc:
        with (
            tc.tile_pool(name="const", bufs=1) as cpool,
            tc.tile_pool(name="w2p", bufs=4) as w2pool,
            tc.tile_pool(name="ps", bufs=8, space="PSUM") as pp,
            tc.tile_pool(name="dram", bufs=1, space="DRAM") as dpool,
        ):
            # ---- warm-up collective: absorbs the one-time cc barrier ---
            dum_in = dpool.tile([1, 1], F32)
            dum_out = dpool.tile([1, 1], F32, addr_space="Shared")
            nc.sync.dma_start(dum_in[:], dum_d[:])
            nc.gpsimd.collective_compute(
                "AllReduce",
                mybir.AluOpType.add,
                replica_groups=RG,
                ins=[dum_in.opt()],
                outs=[dum_out.opt()],
            )

            # ---- small input loads (sync queue) ------------------------
            ada_sb = cpool.tile([128, KA * B], BF16)
            nc.sync.dma_start(ada_sb[:], ada_d[:])
            b1_sb = cpool.tile([128, KH], F32)
            nc.sync.dma_start(b1_sb[:], b1_d[:])
            xs_sb = cpool.tile([B, DS], F32)
            nc.sync.dma_start(xs_sb[:], xs_d[:])
            b2_sb = cpool.tile([1, 2 * ACOLS], BF16)
            nc.sync.dma_start(b2_sb[:], b2_d[:])
            ones_sb = cpool.tile([1, B], BF16)
            nc.sync.dma_start(ones_sb[:], ones_d[:])
            xt_sb = cpool.tile([128, KX * B], BF16)
            nc.sync.dma_start(xt_sb[:], xt_d[:])
            ident_sb = cpool.tile([B, B], BF16)
            nc.sync.dma_start(ident_sb[:], ident_d[:])
            xsb_sb = cpool.tile([B, DS], BF16)
            nc.sync.dma_start(xsb_sb[:], xsb_d[:])

            # ---- w1 (2MB) on the scalar HWDGE queue, 2 chunks ----------
            w1_sb = cpool.tile([128, KH * KA * 128], BF16)
            half = KH * KA * 128 // 2
            nc.scalar.dma_start(w1_sb[:, :half], w1_d[:, :half])
            nc.scalar.dma_start(w1_sb[:, half:], w1_d[:, half:])

            # ---- full h^T, replicated: gelu(w1^T @ ada + b1) -----------
            # m-tile km: hT rows [km*128,(km+1)*128) -> ht_sb[:, km*16:]
            ht_sb = cpool.tile([128, KH * B], BF16)
            for km in range(KH):
                ph = pp.tile([128, B], F32, tag="ps", name=f"ph{km}")
                for k in range(KA):
                    nc.tensor.matmul(
                        ph[:],
                        w1_sb[:, (km * KA + k) * 128 : (km * KA + k + 1) * 128],
                        ada_sb[:, k * B : (k + 1) * B],
                        start=(k == 0),
                        stop=(k == KA - 1),
                    )
                nc.scalar.activation(
                    ht_sb[:, km * B : (km + 1) * B], ph[:],
                    mybir.ActivationFunctionType.Gelu,
                    bias=b1_sb[:, km : km + 1],
                )

            # ---- xw a-half (x_a cols, r-major layout), 4 x 1MB chunks --
            psum_a = [pp.tile([B, 512], F32, tag="ps", name=f"psa{j}") for j in range(4)]
            psum_b = [pp.tile([B, 512], F32, tag="ps", name=f"psb{j}") for j in range(4)]

            for c in range(4):
                w2t = w2pool.tile([128, 2 * ACOLS], BF16, tag="w2", name=f"w2a{c}")
                eng = nc.sync if c % 2 == 0 else nc.scalar
                eng.dma_start(w2t[:], w2_d[:, c * 4096 : (c + 1) * 4096])
                for kk in range(2):
                    k = c * 2 + kk
                    lhs = ht_sb[:, k * B : (k + 1) * B]
                    for n in range(4):
                        nc.tensor.matmul(
                            psum_a[n][:],
                            lhs,
                            w2t[:, kk * ACOLS + n * 512 : kk * ACOLS + (n + 1) * 512],
                            start=(k == 0),
                            stop=False,
                        )
            for n in range(4):  # b2 bias via K=1 ones matmul
                nc.tensor.matmul(
                    psum_a[n][:],
                    ones_sb[:],
                    b2_sb[0:1, n * 512 : (n + 1) * 512],
                    start=False,
                    stop=True,
                )

            # ---- t_partial[b,r] = sum_d xs[b,d] * xw_a[b, r*256+d] -----
            t_sb = cpool.tile([B, RANK], F32)
            tmp_t = cpool.tile([B, 512], F32)
            xs_b2 = xs_sb[:].unsqueeze(1).broadcast_to((B, 2, DS))
            for j in range(4):  # bank j holds r = 2j, 2j+1 (r-major a-layout)
                nc.vector.tensor_tensor(
                    tmp_t[:].rearrange("p (g d) -> p g d", g=2),
                    psum_a[j][:].rearrange("p (g d) -> p g d", g=2),
                    xs_b2,
                    mybir.AluOpType.mult,
                )
                for g in range(2):
                    nc.vector.tensor_reduce(
                        t_sb[:, 2 * j + g : 2 * j + g + 1],
                        tmp_t[:, g * DS : (g + 1) * DS],
                        axis=mybir.AxisListType.X,
                        op=mybir.AluOpType.add,
                    )

            # ---- AllReduce t (overlaps the b-half DMA stream) ----------
            t2_sb = cpool.tile([B, RANK], F32)
            if skip_ar:
                nc.vector.tensor_copy(t2_sb[:], t_sb[:])
            else:
                t_in = dpool.tile([B, RANK], F32)
                t_out = dpool.tile([B, RANK], F32, addr_space="Shared")
                nc.sync.dma_start(t_in[:], t_sb[:])
                nc.gpsimd.collective_compute(
                    "AllReduce",
                    mybir.AluOpType.add,
                    replica_groups=RG,
                    ins=[t_in.opt()],
                    outs=[t_out.opt()],
                )
                nc.sync.dma_start(t2_sb[:], t_out[:])

            # ---- xw b-half (x_b cols, natural o-major) -----------------
            for c in range(4):
                w2t = w2pool.tile([128, 2 * ACOLS], BF16, tag="w2", name=f"w2b{c}")
                eng = nc.sync if c % 2 == 0 else nc.scalar
                eng.dma_start(
                    w2t[:], w2_d[:, KH * ACOLS + c * 4096 : KH * ACOLS + (c + 1) * 4096]
                )
                for kk in range(2):
                    k = c * 2 + kk
                    lhs = ht_sb[:, k * B : (k + 1) * B]
                    for n in range(4):
                        nc.tensor.matmul(
                            psum_b[n][:],
                            lhs,
                            w2t[:, kk * ACOLS + n * 512 : kk * ACOLS + (n + 1) * 512],
                            start=(k == 0),
                            stop=False,
                        )
            for n in range(4):
                nc.tensor.matmul(
                    psum_b[n][:],
                    ones_sb[:],
                    b2_sb[0:1, ACOLS + n * 512 : ACOLS + (n + 1) * 512],
                    start=False,
                    stop=True,
                )

            # ---- stage xw_b out of PSUM (before the AllReduce lands) ---
            xwb_sb = cpool.tile([B, ACOLS], F32)
            for j in range(4):
                nc.vector.tensor_copy(
                    xwb_sb[:, j * 512 : (j + 1) * 512], psum_b[j][:]
                )

            # ---- base term: x @ base[:, o-slice], 1MB chunk ------------
            base_ps = pp.tile([B, DS], F32, tag="ps", name="base_ps")
            bt = w2pool.tile([128, KX * DS], BF16, tag="w2", name="baset")
            nc.scalar.dma_start(bt[:], base_d[:])
            for k in range(KX):
                nc.tensor.matmul(
                    base_ps[:],
                    xt_sb[:, k * B : (k + 1) * B],
                    bt[:, k * DS : (k + 1) * DS],
                    start=(k == 0),
                    stop=False,
                )
            nc.tensor.matmul(  # + residual: I^T @ xs == xs
                base_ps[:], ident_sb[:], xsb_sb[:], start=False, stop=True
            )

            # ---- delta[b,o] = sum_r t[b,r] * xw_b[b, o*8+r] ------------
            delta_sb = cpool.tile([B, DS], F32)
            prod_sb = cpool.tile([B, ACOLS], F32)
            tb = t2_sb[:].unsqueeze(1).broadcast_to((B, DS, RANK))
            nc.vector.tensor_tensor(
                prod_sb[:].rearrange("p (o r) -> p o r", r=RANK),
                xwb_sb[:].rearrange("p (o r) -> p o r", r=RANK),
                tb,
                mybir.AluOpType.mult,
            )
            nc.vector.tensor_reduce(
                delta_sb[:],
                prod_sb[:].rearrange("p (o r) -> p o r", r=RANK),
                axis=mybir.AxisListType.X,
                op=mybir.AluOpType.add,
            )

            # ---- out = (xs + base_term) + delta ------------------------
            out_sb = cpool.tile([B, DS], F32)
            nc.vector.tensor_add(out_sb[:], base_ps[:], delta_sb[:])
            nc.sync.dma_start(out_d[:], out_sb[:])

    nc.compile()
    return nc


def _ktile(a: np.ndarray, p: int = 128) -> np.ndarray:
    """[K*p, m] -> [p, K*m] with free index = k*m + j (k-tile major)."""
    kp, m = a.shape
    k = kp // p
    return np.ascontiguousarray(
        a.reshape(k, p, m).transpose(1, 0, 2).reshape(p, k * m)
    )


def shard_inputs(x, ada_emb, base, w1, b1, w2, b2):
    import ml_dtypes

    bf16 = ml_dtypes.bfloat16
    x = np.asarray(x, np.float32)
    ada_emb = np.asarray(ada_emb, np.float32)
    base = np.asarray(base, np.float32)
    w1 = np.asarray(w1, np.float32)
    b1 = np.asarray(b1, np.float32)
    w2 = np.asarray(w2, bf16)
    b2 = np.asarray(b2, np.float32)

    ada_pre = _ktile(np.ascontiguousarray(ada_emb.T)).astype(bf16)  # [128, 8*16]
    xt_pre = _ktile(np.ascontiguousarray(x.T)).astype(bf16)         # [128, 16*16]
    # w1f[p, (km*KA+k)*128 + m] = w1[k*128+p, km*128+m]
    w1f = np.ascontiguousarray(
        w1.reshape(KA, 128, KH, 128).transpose(1, 2, 0, 3).reshape(128, KH * KA * 128)
    ).astype(bf16)
    b1f = np.ascontiguousarray(b1.reshape(KH, 128).T)              # [128, KH]

    d = np.arange(DS)
    r = np.arange(RANK)
    in_maps = []
    for i in range(NC):
        # x_a columns for this core, permuted r-major: c = r*DS + d
        cols_a = ((i * DS + d)[None, :] * RANK + r[:, None]).reshape(-1)
        w2a = _ktile(w2[:, cols_a])                         # [128, 8*2048]
        w2b = _ktile(w2[:, D * RANK + i * ACOLS : D * RANK + (i + 1) * ACOLS])
        b2a = b2[cols_a]
        b2b = b2[D * RANK + i * ACOLS : D * RANK + (i + 1) * ACOLS]
        in_maps.append({
            "ada": ada_pre,
            "w1f": w1f,
            "b1f": b1f,
            "w2s": np.ascontiguousarray(np.concatenate([w2a, w2b], axis=1)),
            "b2s": np.concatenate([b2a, b2b]).reshape(1, -1).astype(bf16),
            "xts": xt_pre,
            "xs": np.ascontiguousarray(x[:, i * DS : (i + 1) * DS]),
            "ones": np.ones((1, B), bf16),
            "ident": np.eye(B, dtype=bf16),
            "xsb": np.ascontiguousarray(x[:, i * DS : (i + 1) * DS]).astype(bf16),
            "warm": np.zeros((1, 1), np.float32),
            "bases": _ktile(base[:, i * DS : (i + 1) * DS]).astype(bf16),
        })
    return in_maps


def kernel(**inputs) -> np.ndarray:
    global _CACHED_NC
    if _CACHED_NC is None:
        _CACHED_NC = build_nc()
    in_maps = shard_inputs(**inputs)
    res = run_bass_kernel_spmd(_CACHED_NC, in_maps, list(range(NC)))
    return np.concatenate([res.results[i]["out"] for i in range(NC)], axis=1)


if __name__ == "__main__":
    rng = np.random.default_rng(0)
    ins = {
        "x": rng.standard_normal((B, D), np.float32),
        "ada_emb": rng.standard_normal((B, ADA), np.float32),
        "base": rng.standard_normal((D, D), np.float32),
        "w1": rng.standard_normal((ADA, INTER), np.float32) / np.sqrt(ADA),
        "b1": rng.standard_normal((INTER,), np.float32) / np.sqrt(ADA),
        "w2": rng.standard_normal((INTER, D * RANK * 2), np.float32) / np.sqrt(INTER),
        "b2": rng.standard_normal((D * RANK * 2,), np.float32) / np.sqrt(INTER),
    }
    out = kernel(**ins)
    print("out", out.shape, out.dtype, float(np.abs(out).mean()))


# revision 17
# speedup vs baseline: 2.1752x; 2.1752x over previous
"""AdaLoRAWithBase distributed Trainium2 kernel (8 NeuronCores).

Reference computation (B=16, D=2048, ADA=1024, INTER=1024, R=8):
    h   = gelu(ada_emb @ w1 + b1)                  [B, INTER]
    xw  = h @ w2 + b2                              [B, 2*D*R]
    x_a = xw[:, :D*R]  -> [B, D, R]
    x_b = xw[:, D*R:]  -> [B, D, R]
    layer = base + einsum('bdr,bkr->bdk', x_a, x_b)
    out = x + einsum('bc,bco->bo', x, layer)

Key algebra: the B x D x D layer never needs to be materialized:
    out = x + x @ base + sum_r t[:, r] * x_b[:, :, r]
    t[b,r] = sum_d x[b,d] * x_a[b,d,r]

Distribution: RANK == n_cores == 8, so shard by rank r -- core i takes
the x_a and x_b columns of w2 belonging to rank i (stride-8 column
slices, 4096 of the 32768 columns = 1/8 of w2's 128MB). Each core then
computes, fully locally, with NO collectives:
  - h = gelu(ada_emb @ w1 + b1), replicated (w1 is 2MB bf16; measured on
    this runtime any collective costs 50-80us of launch-skew + cc-boot
    barrier, far more than the redundant compute),
  - t_i = sum over ALL d of x[:,d] * x_a[:,d,i]      (its own rank),
  - delta_i = t_i * x_b[:, :, i]                     [B, D], all of D,
  - (x + x @ base)[:, i*256:(i+1)*256]               (its base slice).
Core i's output is delta_i plus its base+residual slice: the output is
SUM-sharded and the host unshards by summing the 8 partials. Since SPMD
cores all run the identical program, the b-half columns are host-rotated
by -i*256 so each core's base slice lands at columns [0,256); the host
un-rotates with np.roll before summing.

All matmul operands are bf16 (1 cycle/row on the PE vs 4 for fp32, and
half the DMA bytes); PSUM accumulation and the t/delta arithmetic stay
fp32. Weight DMAs alternate between the sync and scalar HWDGE queues.
"""

import sys

import numpy as np

for _p in ("/opt/trn_rl_repo",):
    if _p not in sys.path:
        sys.path.insert(0, _p)

from concourse import bacc, bass, mybir, tile
from concourse.bass_utils import run_bass_kernel_spmd

B, D, ADA, INTER, RANK = 16, 2048, 1024, 1024, 8
NC = 8
DS = D // NC          # 256: per-core base/residual o-slice
KA = ADA // 128       # 8 k-tiles for the h matmul
KH = INTER // 128     # 8 k-tiles (and m-tiles) for h / xw
KX = D // 128         # 16 k-tiles for the base matmul
F32 = mybir.dt.float32
BF16 = mybir.dt.bfloat16

_CACHED_NC = None


def build_nc():
    nc = bacc.Bacc(
        "TRN2",
        target_bir_lowering=False,
        debug=False,
        num_devices=NC,
    )

    ada_d = nc.declare_dram_parameter("ada", [128, KA * B], BF16, isOutput=False)
    w1_d = nc.declare_dram_parameter("w1f", [128, KH * KA * 128], BF16, isOutput=False)
    b1_d = nc.declare_dram_parameter("b1f", [128, KH], F32, isOutput=False)
    # per-core: rank-i columns of w2, a-half then b-half, k-tiled
    w2_d = nc.declare_dram_parameter("w2s", [128, KH * 2 * D], BF16, isOutput=False)
    b2_d = nc.declare_dram_parameter("b2s", [1, 2 * D], BF16, isOutput=False)
    xt_d = nc.declare_dram_parameter("xts", [128, KX * B], BF16, isOutput=False)
    xs_d = nc.declare_dram_parameter("xs", [B, D], F32, isOutput=False)
    base_d = nc.declare_dram_parameter("bases", [128, KX * DS], BF16, isOutput=False)
    ones_d = nc.declare_dram_parameter("ones", [1, B], BF16, isOutput=False)
    ident_d = nc.declare_dram_parameter("ident", [B, B], BF16, isOutput=False)
    xsb_d = nc.declare_dram_parameter("xsb", [B, DS], BF16, isOutput=False)
    out_d = nc.declare_dram_parameter("out", [B, D], F32, isOutput=True)

    with tile.TileContext(nc) as tc:
        with (
            tc.tile_pool(name="const", bufs=1) as cpool,
            tc.tile_pool(name="w2p", bufs=4) as w2pool,
            tc.tile_pool(name="ps", bufs=8, space="PSUM") as pp,
        ):
            # ---- small input loads (sync queue) ------------------------
            ada_sb = cpool.tile([128, KA * B], BF16)
            nc.sync.dma_start(ada_sb[:], ada_d[:])
            b1_sb = cpool.tile([128, KH], F32)
            nc.sync.dma_start(b1_sb[:], b1_d[:])
            xs_sb = cpool.tile([B, D], F32)
            nc.sync.dma_start(xs_sb[:], xs_d[:])
            b2_sb = cpool.tile([1, 2 * D], BF16)
            nc.sync.dma_start(b2_sb[:], b2_d[:])
            ones_sb = cpool.tile([1, B], BF16)
            nc.sync.dma_start(ones_sb[:], ones_d[:])
            xt_sb = cpool.tile([128, KX * B], BF16)
            nc.sync.dma_start(xt_sb[:], xt_d[:])
            ident_sb = cpool.tile([B, B], BF16)
            nc.sync.dma_start(ident_sb[:], ident_d[:])
            xsb_sb = cpool.tile([B, DS], BF16)
            nc.sync.dma_start(xsb_sb[:], xsb_d[:])

            # ---- w1 (2MB) on the scalar HWDGE queue, 2 chunks ----------
            w1_sb = cpool.tile([128, KH * KA * 128], BF16)
            half = KH * KA * 128 // 2
            nc.scalar.dma_start(w1_sb[:, :half], w1_d[:, :half])
            nc.scalar.dma_start(w1_sb[:, half:], w1_d[:, half:])

            # ---- full h^T, replicated: gelu(w1^T @ ada + b1) -----------
            # m-tile km: hT rows [km*128,(km+1)*128) -> ht_sb[:, km*16:]
            ht_sb = cpool.tile([128, KH * B], BF16)
            for km in range(KH):
                ph = pp.tile([128, B], F32, tag="ps", name=f"ph{km}")
                for k in range(KA):
                    nc.tensor.matmul(
                        ph[:],
                        w1_sb[:, (km * KA + k) * 128 : (km * KA + k + 1) * 128],
                        ada_sb[:, k * B : (k + 1) * B],
                        start=(k == 0),
                        stop=(k == KA - 1),
                    )
                nc.scalar.activation(
                    ht_sb[:, km * B : (km + 1) * B], ph[:],
                    mybir.ActivationFunctionType.Gelu,
                    bias=b1_sb[:, km : km + 1],
                )

            # ---- xw a-half (rank-i x_a cols, d-ordered), 4 x 1MB chunks
            psum_a = [pp.tile([B, 512], F32, tag="ps", name=f"psa{j}") for j in range(4)]
            psum_b = [pp.tile([B, 512], F32, tag="ps", name=f"psb{j}") for j in range(4)]

            for c in range(4):
                w2t = w2pool.tile([128, 2 * D], BF16, tag="w2", name=f"w2a{c}")
                eng = nc.sync if c % 2 == 0 else nc.scalar
                eng.dma_start(w2t[:], w2_d[:, c * 4096 : (c + 1) * 4096])
                for kk in range(2):
                    k = c * 2 + kk
                    lhs = ht_sb[:, k * B : (k + 1) * B]
                    for n in range(4):
                        nc.tensor.matmul(
                            psum_a[n][:],
                            lhs,
                            w2t[:, kk * D + n * 512 : kk * D + (n + 1) * 512],
                            start=(k == 0),
                            stop=False,
                        )
            for n in range(4):  # b2 bias via K=1 ones matmul
                nc.tensor.matmul(
                    psum_a[n][:],
                    ones_sb[:],
                    b2_sb[0:1, n * 512 : (n + 1) * 512],
                    start=False,
                    stop=True,
                )

            # ---- t_i[b] = sum_d xs[b,d] * xw_a[b,d]  (scalar per b) ----
            tmp_t = cpool.tile([B, D], F32)
            for j in range(4):
                nc.vector.tensor_tensor(
                    tmp_t[:, j * 512 : (j + 1) * 512],
                    psum_a[j][:],
                    xs_sb[:, j * 512 : (j + 1) * 512],
                    mybir.AluOpType.mult,
                )
            t_sc = cpool.tile([B, 1], F32)
            nc.vector.tensor_reduce(
                t_sc[:], tmp_t[:],
                axis=mybir.AxisListType.X, op=mybir.AluOpType.add,
            )

            # ---- xw b-half (rank-i x_b cols, rotated o-order) ----------
            for c in range(4):
                w2t = w2pool.tile([128, 2 * D], BF16, tag="w2", name=f"w2b{c}")
                eng = nc.sync if c % 2 == 0 else nc.scalar
                eng.dma_start(
                    w2t[:], w2_d[:, KH * D + c * 4096 : KH * D + (c + 1) * 4096]
                )
                for kk in range(2):
                    k = c * 2 + kk
                    lhs = ht_sb[:, k * B : (k + 1) * B]
                    for n in range(4):
                        nc.tensor.matmul(
                            psum_b[n][:],
                            lhs,
                            w2t[:, kk * D + n * 512 : kk * D + (n + 1) * 512],
                            start=(k == 0),
                            stop=False,
                        )
            for n in range(4):
                nc.tensor.matmul(
                    psum_b[n][:],
                    ones_sb[:],
                    b2_sb[0:1, D + n * 512 : D + (n + 1) * 512],
                    start=False,
                    stop=True,
                )

            # ---- base term + residual: x @ base[:, slice] + xs ---------
            base_ps = pp.tile([B, DS], F32, tag="ps", name="base_ps")
            bt = w2pool.tile([128, KX * DS], BF16, tag="w2", name="baset")
            nc.scalar.dma_start(bt[:], base_d[:])
            for k in range(KX):
                nc.tensor.matmul(
                    base_ps[:],
                    xt_sb[:, k * B : (k + 1) * B],
                    bt[:, k * DS : (k + 1) * DS],
                    start=(k == 0),
                    stop=False,
                )
            nc.tensor.matmul(  # + residual: I^T @ xs == xs
                base_ps[:], ident_sb[:], xsb_sb[:], start=False, stop=True
            )

            # ---- out partial = t_i * xw_b  (+ base/residual at [0,DS)) -
            out_sb = cpool.tile([B, D], F32)
            for j in range(4):
                nc.vector.tensor_scalar_mul(
                    out_sb[:, j * 512 : (j + 1) * 512],
                    psum_b[j][:],
                    t_sc[:, 0:1],
                )
            nc.vector.tensor_tensor(
                out_sb[:, 0:DS], out_sb[:, 0:DS], base_ps[:],
                mybir.AluOpType.add,
            )
            nc.sync.dma_start(out_d[:], out_sb[:])

    nc.compile()
    return nc


def _ktile(a: np.ndarray, p: int = 128) -> np.ndarray:
    """[K*p, m] -> [p, K*m] with free index = k*m + j (k-tile major)."""
    kp, m = a.shape
    k = kp // p
    return np.ascontiguousarray(
        a.reshape(k, p, m).transpose(1, 0, 2).reshape(p, k * m)
    )


def shard_inputs(x, ada_emb, base, w1, b1, w2, b2):
    import ml_dtypes

    bf16 = ml_dtypes.bfloat16
    x = np.ascontiguousarray(np.asarray(x, np.float32))
    ada_emb = np.asarray(ada_emb, np.float32)
    base = np.asarray(base, np.float32)
    w1 = np.asarray(w1, np.float32)
    b1 = np.asarray(b1, np.float32)
    w2 = np.asarray(w2, bf16)
    b2 = np.asarray(b2, np.float32)

    ada_pre = _ktile(np.ascontiguousarray(ada_emb.T)).astype(bf16)  # [128, 8*16]
    xt_pre = _ktile(np.ascontiguousarray(x.T)).astype(bf16)         # [128, 16*16]
    # w1f[p, (km*KA+k)*128 + m] = w1[k*128+p, km*128+m]
    w1f = np.ascontiguousarray(
        w1.reshape(KA, 128, KH, 128).transpose(1, 2, 0, 3).reshape(128, KH * KA * 128)
    ).astype(bf16)
    b1f = np.ascontiguousarray(b1.reshape(KH, 128).T)              # [128, KH]

    w2a, w2b = w2[:, : D * RANK], w2[:, D * RANK :]
    b2a, b2b = b2[: D * RANK], b2[D * RANK :]
    in_maps = []
    for i in range(NC):
        # rank-i columns: stride-RANK slices; b-half rotated by -i*DS so
        # the base/residual slice lands at output columns [0, DS)
        w2ai = _ktile(np.ascontiguousarray(w2a[:, i::RANK]))
        w2bi = _ktile(np.roll(w2b[:, i::RANK], -i * DS, axis=1))
        in_maps.append({
            "ada": ada_pre,
            "w1f": w1f,
            "b1f": b1f,
            "w2s": np.ascontiguousarray(np.concatenate([w2ai, w2bi], axis=1)),
            "b2s": np.concatenate(
                [b2a[i::RANK], np.roll(b2b[i::RANK], -i * DS)]
            ).reshape(1, -1).astype(bf16),
            "xts": xt_pre,
            "xs": x,
            "ones": np.ones((1, B), bf16),
            "ident": np.eye(B, dtype=bf16),
            "xsb": np.ascontiguousarray(x[:, i * DS : (i + 1) * DS]).astype(bf16),
            "bases": _ktile(base[:, i * DS : (i + 1) * DS]).astype(bf16),
        })
    return in_maps


def kernel(**inputs) -> np.ndarray:
    global _CACHED_NC
    if _CACHED_NC is None:
        _CACHED_NC = build_nc()
    in_maps = shard_inputs(**inputs)
    res = run_bass_kernel_spmd(_CACHED_NC, in_maps, list(range(NC)))
    # Each core's "out" is a sum-shard of the output, column-rotated by
    # -i*DS. Un-rotate and sum to unshard.
    total = np.zeros((B, D), np.float32)
    for i in range(NC):
        total += np.roll(res.results[i]["out"], i * DS, axis=1)
    return total


if __name__ == "__main__":
    rng = np.random.default_rng(0)
    ins = {
        "x": rng.standard_normal((B, D), np.float32),
        "ada_emb": rng.standard_normal((B, ADA), np.float32),
        "base": rng.standard_normal((D, D), np.float32),
        "w1": rng.standard_normal((ADA, INTER), np.float32) / np.sqrt(ADA),
        "b1": rng.standard_normal((INTER,), np.float32) / np.sqrt(ADA),
        "w2": rng.standard_normal((INTER, D * RANK * 2), np.float32) / np.sqrt(INTER),
        "b2": rng.standard_normal((D * RANK * 2,), np.float32) / np.sqrt(INTER),
    }
    out = kernel(**ins)
    print("out", out.shape, out.dtype, float(np.abs(out).mean()))


# revision 18
# speedup vs baseline: 2.2203x; 1.0207x over previous
"""AdaLoRAWithBase distributed Trainium2 kernel (8 NeuronCores).

Reference computation (B=16, D=2048, ADA=1024, INTER=1024, R=8):
    h   = gelu(ada_emb @ w1 + b1)                  [B, INTER]
    xw  = h @ w2 + b2                              [B, 2*D*R]
    x_a = xw[:, :D*R]  -> [B, D, R]
    x_b = xw[:, D*R:]  -> [B, D, R]
    layer = base + einsum('bdr,bkr->bdk', x_a, x_b)
    out = x + einsum('bc,bco->bo', x, layer)

Key algebra: the B x D x D layer never needs to be materialized:
    out = x + x @ base + sum_r t[:, r] * x_b[:, :, r]
    t[b,r] = sum_d x[b,d] * x_a[b,d,r]

Distribution: RANK == n_cores == 8, so shard by rank r -- core i takes
the x_a and x_b columns of w2 belonging to rank i (stride-8 column
slices, 4096 of the 32768 columns = 1/8 of w2's 128MB). Each core then
computes, fully locally, with NO collectives:
  - h = gelu(ada_emb @ w1 + b1), replicated (w1 is 2MB bf16; measured on
    this runtime any collective costs 50-80us of launch-skew + cc-boot
    barrier, far more than the redundant compute),
  - t_i = sum over ALL d of x[:,d] * x_a[:,d,i]      (its own rank),
  - delta_i = t_i * x_b[:, :, i]                     [B, D], all of D,
  - (x + x @ base)[:, i*256:(i+1)*256]               (its base slice).
Core i's output is delta_i plus its base+residual slice: the output is
SUM-sharded and the host unshards by summing the 8 partials. Since SPMD
cores all run the identical program, the b-half columns are host-rotated
by -i*256 so each core's base slice lands at columns [0,256); the host
un-rotates with np.roll before summing.

All matmul operands are bf16 (1 cycle/row on the PE vs 4 for fp32, and
half the DMA bytes); PSUM accumulation and the t/delta arithmetic stay
fp32. Weight DMAs alternate between the sync and scalar HWDGE queues.
"""

import sys

import numpy as np

for _p in ("/opt/trn_rl_repo",):
    if _p not in sys.path:
        sys.path.insert(0, _p)

from concourse import bacc, bass, mybir, tile
from concourse.bass_utils import run_bass_kernel_spmd

B, D, ADA, INTER, RANK = 16, 2048, 1024, 1024, 8
NC = 8
DS = D // NC          # 256: per-core base/residual o-slice
KA = ADA // 128       # 8 k-tiles for the h matmul
KH = INTER // 128     # 8 k-tiles (and m-tiles) for h / xw
KX = D // 128         # 16 k-tiles for the base matmul
F32 = mybir.dt.float32
BF16 = mybir.dt.bfloat16

_CACHED_NC = None


def build_nc():
    nc = bacc.Bacc(
        "TRN2",
        target_bir_lowering=False,
        debug=False,
        num_devices=NC,
    )

    ada_d = nc.declare_dram_parameter("ada", [128, KA * B], BF16, isOutput=False)
    w1_d = nc.declare_dram_parameter("w1f", [128, KH * KA * 128], BF16, isOutput=False)
    b1_d = nc.declare_dram_parameter("b1f", [128, KH], F32, isOutput=False)
    # per-core: rank-i columns of w2, a-half then b-half, k-tiled
    w2_d = nc.declare_dram_parameter("w2s", [128, KH * 2 * D], BF16, isOutput=False)
    b2_d = nc.declare_dram_parameter("b2s", [1, 2 * D], BF16, isOutput=False)
    xt_d = nc.declare_dram_parameter("xts", [128, KX * B], BF16, isOutput=False)
    xs_d = nc.declare_dram_parameter("xs", [B, D], F32, isOutput=False)
    base_d = nc.declare_dram_parameter("bases", [128, KX * DS], BF16, isOutput=False)
    ones_d = nc.declare_dram_parameter("ones", [1, B], BF16, isOutput=False)
    ident_d = nc.declare_dram_parameter("ident", [B, B], BF16, isOutput=False)
    xsb_d = nc.declare_dram_parameter("xsb", [B, DS], BF16, isOutput=False)
    out_d = nc.declare_dram_parameter("out", [B, D], F32, isOutput=True)

    with tile.TileContext(nc) as tc:
        with (
            tc.tile_pool(name="const", bufs=1) as cpool,
            tc.tile_pool(name="w2p", bufs=6) as w2pool,
            tc.tile_pool(name="ps", bufs=8, space="PSUM") as pp,
        ):
            # ---- small input loads (sync queue) ------------------------
            ada_sb = cpool.tile([128, KA * B], BF16)
            nc.sync.dma_start(ada_sb[:], ada_d[:])
            b1_sb = cpool.tile([128, KH], F32)
            nc.sync.dma_start(b1_sb[:], b1_d[:])
            xs_sb = cpool.tile([B, D], F32)
            nc.sync.dma_start(xs_sb[:], xs_d[:])
            b2_sb = cpool.tile([1, 2 * D], BF16)
            nc.sync.dma_start(b2_sb[:], b2_d[:])
            ones_sb = cpool.tile([1, B], BF16)
            nc.sync.dma_start(ones_sb[:], ones_d[:])
            xt_sb = cpool.tile([128, KX * B], BF16)
            nc.sync.dma_start(xt_sb[:], xt_d[:])
            ident_sb = cpool.tile([B, B], BF16)
            nc.sync.dma_start(ident_sb[:], ident_d[:])
            xsb_sb = cpool.tile([B, DS], BF16)
            nc.sync.dma_start(xsb_sb[:], xsb_d[:])

            # ---- w1 (2MB) on the scalar HWDGE queue, 2 chunks ----------
            w1_sb = cpool.tile([128, KH * KA * 128], BF16)
            half = KH * KA * 128 // 2
            nc.gpsimd.dma_start(w1_sb[:, :half], w1_d[:, :half])
            nc.gpsimd.dma_start(w1_sb[:, half:], w1_d[:, half:])

            # ---- full h^T, replicated: gelu(w1^T @ ada + b1) -----------
            # m-tile km: hT rows [km*128,(km+1)*128) -> ht_sb[:, km*16:]
            ht_sb = cpool.tile([128, KH * B], BF16)
            for km in range(KH):
                ph = pp.tile([128, B], F32, tag="ps", name=f"ph{km}")
                for k in range(KA):
                    nc.tensor.matmul(
                        ph[:],
                        w1_sb[:, (km * KA + k) * 128 : (km * KA + k + 1) * 128],
                        ada_sb[:, k * B : (k + 1) * B],
                        start=(k == 0),
                        stop=(k == KA - 1),
                    )
                nc.scalar.activation(
                    ht_sb[:, km * B : (km + 1) * B], ph[:],
                    mybir.ActivationFunctionType.Gelu,
                    bias=b1_sb[:, km : km + 1],
                )

            # ---- xw a-half (rank-i x_a cols, d-ordered), 4 x 1MB chunks
            psum_a = [pp.tile([B, 512], F32, tag="ps", name=f"psa{j}") for j in range(4)]
            psum_b = [pp.tile([B, 512], F32, tag="ps", name=f"psb{j}") for j in range(4)]

            for c in range(4):
                w2t = w2pool.tile([128, 2 * D], BF16, tag="w2", name=f"w2a{c}")
                eng = nc.sync if c % 2 == 0 else nc.gpsimd
                eng.dma_start(w2t[:], w2_d[:, c * 4096 : (c + 1) * 4096])
                for kk in range(2):
                    k = c * 2 + kk
                    lhs = ht_sb[:, k * B : (k + 1) * B]
                    for n in range(4):
                        nc.tensor.matmul(
                            psum_a[n][:],
                            lhs,
                            w2t[:, kk * D + n * 512 : kk * D + (n + 1) * 512],
                            start=(k == 0),
                            stop=False,
                        )
            for n in range(4):  # b2 bias via K=1 ones matmul
                nc.tensor.matmul(
                    psum_a[n][:],
                    ones_sb[:],
                    b2_sb[0:1, n * 512 : (n + 1) * 512],
                    start=False,
                    stop=True,
                )

            # ---- t_i[b] = sum_d xs[b,d] * xw_a[b,d]  (scalar per b) ----
            tmp_t = cpool.tile([B, D], F32)
            for j in range(4):
                nc.vector.tensor_tensor(
                    tmp_t[:, j * 512 : (j + 1) * 512],
                    psum_a[j][:],
                    xs_sb[:, j * 512 : (j + 1) * 512],
                    mybir.AluOpType.mult,
                )
            t_sc = cpool.tile([B, 1], F32)
            nc.vector.tensor_reduce(
                t_sc[:], tmp_t[:],
                axis=mybir.AxisListType.X, op=mybir.AluOpType.add,
            )

            # ---- xw b-half (rank-i x_b cols, rotated o-order) ----------
            for c in range(4):
                w2t = w2pool.tile([128, 2 * D], BF16, tag="w2", name=f"w2b{c}")
                eng = nc.sync if c % 2 == 0 else nc.gpsimd
                eng.dma_start(
                    w2t[:], w2_d[:, KH * D + c * 4096 : KH * D + (c + 1) * 4096]
                )
                for kk in range(2):
                    k = c * 2 + kk
                    lhs = ht_sb[:, k * B : (k + 1) * B]
                    for n in range(4):
                        nc.tensor.matmul(
                            psum_b[n][:],
                            lhs,
                            w2t[:, kk * D + n * 512 : kk * D + (n + 1) * 512],
                            start=(k == 0),
                            stop=False,
                        )
            for n in range(4):
                nc.tensor.matmul(
                    psum_b[n][:],
                    ones_sb[:],
                    b2_sb[0:1, D + n * 512 : D + (n + 1) * 512],
                    start=False,
                    stop=True,
                )

            # ---- base term + residual: x @ base[:, slice] + xs ---------
            base_ps = pp.tile([B, DS], F32, tag="ps", name="base_ps")
            bt = w2pool.tile([128, KX * DS], BF16, tag="w2", name="baset")
            nc.sync.dma_start(bt[:], base_d[:])
            for k in range(KX):
                nc.tensor.matmul(
                    base_ps[:],
                    xt_sb[:, k * B : (k + 1) * B],
                    bt[:, k * DS : (k + 1) * DS],
                    start=(k == 0),
                    stop=False,
                )
            nc.tensor.matmul(  # + residual: I^T @ xs == xs
                base_ps[:], ident_sb[:], xsb_sb[:], start=False, stop=True
            )

            # ---- out partial = t_i * xw_b  (+ base/residual at [0,DS)) -
            out_sb = cpool.tile([B, D], F32)
            for j in range(4):
                nc.vector.tensor_scalar_mul(
                    out_sb[:, j * 512 : (j + 1) * 512],
                    psum_b[j][:],
                    t_sc[:, 0:1],
                )
            nc.vector.tensor_tensor(
                out_sb[:, 0:DS], out_sb[:, 0:DS], base_ps[:],
                mybir.AluOpType.add,
            )
            nc.sync.dma_start(out_d[:], out_sb[:])

    nc.compile()
    return nc


def _ktile(a: np.ndarray, p: int = 128) -> np.ndarray:
    """[K*p, m] -> [p, K*m] with free index = k*m + j (k-tile major)."""
    kp, m = a.shape
    k = kp // p
    return np.ascontiguousarray(
        a.reshape(k, p, m).transpose(1, 0, 2).reshape(p, k * m)
    )


def shard_inputs(x, ada_emb, base, w1, b1, w2, b2):
    import ml_dtypes

    bf16 = ml_dtypes.bfloat16
    x = np.ascontiguousarray(np.asarray(x, np.float32))
    ada_emb = np.asarray(ada_emb, np.float32)
    base = np.asarray(base, np.float32)
    w1 = np.asarray(w1, np.float32)
    b1 = np.asarray(b1, np.float32)
    w2 = np.asarray(w2, bf16)
    b2 = np.asarray(b2, np.float32)

    ada_pre = _ktile(np.ascontiguousarray(ada_emb.T)).astype(bf16)  # [128, 8*16]
    xt_pre = _ktile(np.ascontiguousarray(x.T)).astype(bf16)         # [128, 16*16]
    # w1f[p, (km*KA+k)*128 + m] = w1[k*128+p, km*128+m]
    w1f = np.ascontiguousarray(
        w1.reshape(KA, 128, KH, 128).transpose(1, 2, 0, 3).reshape(128, KH * KA * 128)
    ).astype(bf16)
    b1f = np.ascontiguousarray(b1.reshape(KH, 128).T)              # [128, KH]

    w2a, w2b = w2[:, : D * RANK], w2[:, D * RANK :]
    b2a, b2b = b2[: D * RANK], b2[D * RANK :]
    in_maps = []
    for i in range(NC):
        # rank-i columns: stride-RANK slices; b-half rotated by -i*DS so
        # the base/residual slice lands at output columns [0, DS)
        w2ai = _ktile(np.ascontiguousarray(w2a[:, i::RANK]))
        w2bi = _ktile(np.roll(w2b[:, i::RANK], -i * DS, axis=1))
        in_maps.append({
            "ada": ada_pre,
            "w1f": w1f,
            "b1f": b1f,
            "w2s": np.ascontiguousarray(np.concatenate([w2ai, w2bi], axis=1)),
            "b2s": np.concatenate(
                [b2a[i::RANK], np.roll(b2b[i::RANK], -i * DS)]
            ).reshape(1, -1).astype(bf16),
            "xts": xt_pre,
            "xs": x,
            "ones": np.ones((1, B), bf16),
            "ident": np.eye(B, dtype=bf16),
            "xsb": np.ascontiguousarray(x[:, i * DS : (i + 1) * DS]).astype(bf16),
            "bases": _ktile(base[:, i * DS : (i + 1) * DS]).astype(bf16),
        })
    return in_maps


def kernel(**inputs) -> np.ndarray:
    global _CACHED_NC
    if _CACHED_NC is None:
        _CACHED_NC = build_nc()
    in_maps = shard_inputs(**inputs)
    res = run_bass_kernel_spmd(_CACHED_NC, in_maps, list(range(NC)))
    # Each core's "out" is a sum-shard of the output, column-rotated by
    # -i*DS. Un-rotate and sum to unshard.
    total = np.zeros((B, D), np.float32)
    for i in range(NC):
        total += np.roll(res.results[i]["out"], i * DS, axis=1)
    return total


if __name__ == "__main__":
    rng = np.random.default_rng(0)
    ins = {
        "x": rng.standard_normal((B, D), np.float32),
        "ada_emb": rng.standard_normal((B, ADA), np.float32),
        "base": rng.standard_normal((D, D), np.float32),
        "w1": rng.standard_normal((ADA, INTER), np.float32) / np.sqrt(ADA),
        "b1": rng.standard_normal((INTER,), np.float32) / np.sqrt(ADA),
        "w2": rng.standard_normal((INTER, D * RANK * 2), np.float32) / np.sqrt(INTER),
        "b2": rng.standard_normal((D * RANK * 2,), np.float32) / np.sqrt(INTER),
    }
    out = kernel(**ins)
    print("out", out.shape, out.dtype, float(np.abs(out).mean()))


# revision 19
# speedup vs baseline: 2.3485x; 1.0577x over previous
"""AdaLoRAWithBase distributed Trainium2 kernel (8 NeuronCores).

Reference computation (B=16, D=2048, ADA=1024, INTER=1024, R=8):
    h   = gelu(ada_emb @ w1 + b1)                  [B, INTER]
    xw  = h @ w2 + b2                              [B, 2*D*R]
    x_a = xw[:, :D*R]  -> [B, D, R]
    x_b = xw[:, D*R:]  -> [B, D, R]
    layer = base + einsum('bdr,bkr->bdk', x_a, x_b)
    out = x + einsum('bc,bco->bo', x, layer)

Key algebra: the B x D x D layer never needs to be materialized:
    out = x + x @ base + sum_r t[:, r] * x_b[:, :, r]
    t[b,r] = sum_d x[b,d] * x_a[b,d,r]

Distribution: RANK == n_cores == 8, so shard by rank r -- core i takes
the x_a and x_b columns of w2 belonging to rank i (stride-8 column
slices, 4096 of the 32768 columns = 1/8 of w2's 128MB). Each core then
computes, fully locally, with NO collectives:
  - h = gelu(ada_emb @ w1 + b1), replicated (w1 is 2MB bf16; measured on
    this runtime any collective costs 50-80us of launch-skew + cc-boot
    barrier, far more than the redundant compute),
  - t_i = sum over ALL d of x[:,d] * x_a[:,d,i]      (its own rank),
  - delta_i = t_i * x_b[:, :, i]                     [B, D], all of D,
  - (x + x @ base)[:, i*256:(i+1)*256]               (its base slice).
Core i's output is delta_i plus its base+residual slice: the output is
SUM-sharded and the host unshards by summing the 8 partials. Since SPMD
cores all run the identical program, the b-half columns are host-rotated
by -i*256 so each core's base slice lands at columns [0,256); the host
un-rotates with np.roll before summing.

All matmul operands are bf16 (1 cycle/row on the PE vs 4 for fp32, and
half the DMA bytes); PSUM accumulation and the t/delta arithmetic stay
fp32. Weight DMAs alternate between the sync and scalar HWDGE queues.
"""

import sys

import numpy as np

for _p in ("/opt/trn_rl_repo",):
    if _p not in sys.path:
        sys.path.insert(0, _p)

from concourse import bacc, bass, mybir, tile
from concourse.bass_utils import run_bass_kernel_spmd

B, D, ADA, INTER, RANK = 16, 2048, 1024, 1024, 8
NC = 8
DS = D // NC          # 256: per-core base/residual o-slice
KA = ADA // 128       # 8 k-tiles for the h matmul
KH = INTER // 128     # 8 k-tiles (and m-tiles) for h / xw
KX = D // 128         # 16 k-tiles for the base matmul
F32 = mybir.dt.float32
BF16 = mybir.dt.bfloat16

_CACHED_NC = None


def build_nc():
    nc = bacc.Bacc(
        "TRN2",
        target_bir_lowering=False,
        debug=False,
        num_devices=NC,
    )

    ada_d = nc.declare_dram_parameter("ada", [128, KA * B], BF16, isOutput=False)
    w1_d = nc.declare_dram_parameter("w1f", [128, KH * KA * 128], BF16, isOutput=False)
    b1_d = nc.declare_dram_parameter("b1f", [128, KH], F32, isOutput=False)
    # per-core: rank-i columns of w2, a-half then b-half, k-tiled
    w2_d = nc.declare_dram_parameter("w2s", [128, KH * 2 * D], BF16, isOutput=False)
    b2_d = nc.declare_dram_parameter("b2s", [1, 2 * D], BF16, isOutput=False)
    xt_d = nc.declare_dram_parameter("xts", [128, KX * B], BF16, isOutput=False)
    xs_d = nc.declare_dram_parameter("xs", [B, D], F32, isOutput=False)
    base_d = nc.declare_dram_parameter("bases", [128, KX * DS], BF16, isOutput=False)
    ones_d = nc.declare_dram_parameter("ones", [1, B], BF16, isOutput=False)
    ident_d = nc.declare_dram_parameter("ident", [B, B], BF16, isOutput=False)
    xsb_d = nc.declare_dram_parameter("xsb", [B, DS], BF16, isOutput=False)
    out_d = nc.declare_dram_parameter("out", [B, D], F32, isOutput=True)

    with tile.TileContext(nc) as tc:
        with (
            tc.tile_pool(name="const", bufs=1) as cpool,
            tc.tile_pool(name="w2p", bufs=6) as w2pool,
            tc.tile_pool(name="ps", bufs=8, space="PSUM") as pp,
        ):
            # ---- small input loads (sync queue) ------------------------
            ada_sb = cpool.tile([128, KA * B], BF16)
            nc.sync.dma_start(ada_sb[:], ada_d[:])
            b1_sb = cpool.tile([128, KH], F32)
            nc.sync.dma_start(b1_sb[:], b1_d[:])
            xs_sb = cpool.tile([B, D], F32)
            nc.sync.dma_start(xs_sb[:], xs_d[:])
            b2_sb = cpool.tile([1, 2 * D], BF16)
            nc.sync.dma_start(b2_sb[:], b2_d[:])
            ones_sb = cpool.tile([1, B], BF16)
            nc.sync.dma_start(ones_sb[:], ones_d[:])
            xt_sb = cpool.tile([128, KX * B], BF16)
            nc.sync.dma_start(xt_sb[:], xt_d[:])
            ident_sb = cpool.tile([B, B], BF16)
            nc.sync.dma_start(ident_sb[:], ident_d[:])
            xsb_sb = cpool.tile([B, DS], BF16)
            nc.sync.dma_start(xsb_sb[:], xsb_d[:])

            # ---- w1 (2MB) on the scalar HWDGE queue, 2 chunks ----------
            w1_sb = cpool.tile([128, KH * KA * 128], BF16)
            half = KH * KA * 128 // 2
            nc.gpsimd.dma_start(w1_sb[:, :half], w1_d[:, :half])
            nc.gpsimd.dma_start(w1_sb[:, half:], w1_d[:, half:])

            # ---- full h^T, replicated: gelu(w1^T @ ada + b1) -----------
            # m-tile km: hT rows [km*128,(km+1)*128) -> ht_sb[:, km*16:]
            ht_sb = cpool.tile([128, KH * B], BF16)
            for km in range(KH):
                ph = pp.tile([128, B], F32, tag="ps", name=f"ph{km}")
                for k in range(KA):
                    nc.tensor.matmul(
                        ph[:],
                        w1_sb[:, (km * KA + k) * 128 : (km * KA + k + 1) * 128],
                        ada_sb[:, k * B : (k + 1) * B],
                        start=(k == 0),
                        stop=(k == KA - 1),
                    )
                nc.scalar.activation(
                    ht_sb[:, km * B : (km + 1) * B], ph[:],
                    mybir.ActivationFunctionType.Gelu,
                    bias=b1_sb[:, km : km + 1],
                )

            # ---- base term + residual: x @ base[:, slice] + xs ---------
            base_ps = pp.tile([B, DS], F32, tag="ps", name="base_ps")
            bt = w2pool.tile([128, KX * DS], BF16, tag="w2", name="baset")
            nc.sync.dma_start(bt[:], base_d[:])
            for k in range(KX):
                nc.tensor.matmul(
                    base_ps[:],
                    xt_sb[:, k * B : (k + 1) * B],
                    bt[:, k * DS : (k + 1) * DS],
                    start=(k == 0),
                    stop=False,
                )
            nc.tensor.matmul(  # + residual: I^T @ xs == xs
                base_ps[:], ident_sb[:], xsb_sb[:], start=False, stop=True
            )

            # ---- xw a-half: n-major 1MB chunks, one PSUM bank each -----
            # chunk nb holds all 8 k-tiles for d-columns [nb*512,(nb+1)*512)
            psum_a = [pp.tile([B, 512], F32, tag="ps", name=f"psa{j}") for j in range(4)]
            psum_b = [pp.tile([B, 512], F32, tag="ps", name=f"psb{j}") for j in range(4)]
            tmp_t = cpool.tile([B, D], F32)
            t4 = cpool.tile([B, 4], F32)
            for nb in range(4):
                w2t = w2pool.tile([128, 2 * D], BF16, tag="w2", name=f"w2a{nb}")
                eng = nc.sync if nb % 2 == 0 else nc.gpsimd
                eng.dma_start(w2t[:], w2_d[:, nb * 4096 : (nb + 1) * 4096])
                for k in range(KH):
                    nc.tensor.matmul(
                        psum_a[nb][:],
                        ht_sb[:, k * B : (k + 1) * B],
                        w2t[:, k * 512 : (k + 1) * 512],
                        start=(k == 0),
                        stop=False,
                    )
                nc.tensor.matmul(  # b2 bias via K=1 ones matmul
                    psum_a[nb][:],
                    ones_sb[:],
                    b2_sb[0:1, nb * 512 : (nb + 1) * 512],
                    start=False,
                    stop=True,
                )
                # partial t for this bank, pipelined with the stream
                nc.vector.tensor_tensor(
                    tmp_t[:, nb * 512 : (nb + 1) * 512],
                    psum_a[nb][:],
                    xs_sb[:, nb * 512 : (nb + 1) * 512],
                    mybir.AluOpType.mult,
                )
                nc.vector.tensor_reduce(
                    t4[:, nb : nb + 1],
                    tmp_t[:, nb * 512 : (nb + 1) * 512],
                    axis=mybir.AxisListType.X, op=mybir.AluOpType.add,
                )
            t_sc = cpool.tile([B, 1], F32)
            nc.vector.tensor_reduce(
                t_sc[:], t4[:],
                axis=mybir.AxisListType.X, op=mybir.AluOpType.add,
            )

            # ---- xw b-half: n-major chunks, out muls pipelined ---------
            out_sb = cpool.tile([B, D], F32)
            for nb in range(4):
                w2t = w2pool.tile([128, 2 * D], BF16, tag="w2", name=f"w2b{nb}")
                eng = nc.sync if nb % 2 == 0 else nc.gpsimd
                eng.dma_start(
                    w2t[:], w2_d[:, KH * D + nb * 4096 : KH * D + (nb + 1) * 4096]
                )
                for k in range(KH):
                    nc.tensor.matmul(
                        psum_b[nb][:],
                        ht_sb[:, k * B : (k + 1) * B],
                        w2t[:, k * 512 : (k + 1) * 512],
                        start=(k == 0),
                        stop=False,
                    )
                nc.tensor.matmul(
                    psum_b[nb][:],
                    ones_sb[:],
                    b2_sb[0:1, D + nb * 512 : D + (nb + 1) * 512],
                    start=False,
                    stop=True,
                )
                nc.vector.tensor_scalar_mul(
                    out_sb[:, nb * 512 : (nb + 1) * 512],
                    psum_b[nb][:],
                    t_sc[:, 0:1],
                )
                if nb == 0:  # base+residual lands in columns [0, DS)
                    nc.vector.tensor_tensor(
                        out_sb[:, 0:DS], out_sb[:, 0:DS], base_ps[:],
                        mybir.AluOpType.add,
                    )
            nc.sync.dma_start(out_d[:], out_sb[:])

    nc.compile()
    return nc


def _ktile(a: np.ndarray, p: int = 128) -> np.ndarray:
    """[K*p, m] -> [p, K*m] with free index = k*m + j (k-tile major)."""
    kp, m = a.shape
    k = kp // p
    return np.ascontiguousarray(
        a.reshape(k, p, m).transpose(1, 0, 2).reshape(p, k * m)
    )


def shard_inputs(x, ada_emb, base, w1, b1, w2, b2):
    import ml_dtypes

    bf16 = ml_dtypes.bfloat16
    x = np.ascontiguousarray(np.asarray(x, np.float32))
    ada_emb = np.asarray(ada_emb, np.float32)
    base = np.asarray(base, np.float32)
    w1 = np.asarray(w1, np.float32)
    b1 = np.asarray(b1, np.float32)
    w2 = np.asarray(w2, bf16)
    b2 = np.asarray(b2, np.float32)

    ada_pre = _ktile(np.ascontiguousarray(ada_emb.T)).astype(bf16)  # [128, 8*16]
    xt_pre = _ktile(np.ascontiguousarray(x.T)).astype(bf16)         # [128, 16*16]
    # w1f[p, (km*KA+k)*128 + m] = w1[k*128+p, km*128+m]
    w1f = np.ascontiguousarray(
        w1.reshape(KA, 128, KH, 128).transpose(1, 2, 0, 3).reshape(128, KH * KA * 128)
    ).astype(bf16)
    b1f = np.ascontiguousarray(b1.reshape(KH, 128).T)              # [128, KH]

    w2a, w2b = w2[:, : D * RANK], w2[:, D * RANK :]
    b2a, b2b = b2[: D * RANK], b2[D * RANK :]
    in_maps = []
    for i in range(NC):
        # rank-i columns: stride-RANK slices; b-half rotated by -i*DS so
        # the base/residual slice lands at output columns [0, DS)
        # n-major blocks: free idx = nb*(8*512) + k*512 + c
        def _nmajor(a):
            return np.ascontiguousarray(
                a.reshape(KH, 128, 4, 512).transpose(1, 2, 0, 3).reshape(128, KH * 2048)
            )
        w2ai = _nmajor(np.ascontiguousarray(w2a[:, i::RANK]))
        w2bi = _nmajor(np.ascontiguousarray(np.roll(w2b[:, i::RANK], -i * DS, axis=1)))
        in_maps.append({
            "ada": ada_pre,
            "w1f": w1f,
            "b1f": b1f,
            "w2s": np.ascontiguousarray(np.concatenate([w2ai, w2bi], axis=1)),
            "b2s": np.concatenate(
                [b2a[i::RANK], np.roll(b2b[i::RANK], -i * DS)]
            ).reshape(1, -1).astype(bf16),
            "xts": xt_pre,
            "xs": x,
            "ones": np.ones((1, B), bf16),
            "ident": np.eye(B, dtype=bf16),
            "xsb": np.ascontiguousarray(x[:, i * DS : (i + 1) * DS]).astype(bf16),
            "bases": _ktile(base[:, i * DS : (i + 1) * DS]).astype(bf16),
        })
    return in_maps


def kernel(**inputs) -> np.ndarray:
    global _CACHED_NC
    if _CACHED_NC is None:
        _CACHED_NC = build_nc()
    in_maps = shard_inputs(**inputs)
    res = run_bass_kernel_spmd(_CACHED_NC, in_maps, list(range(NC)))
    # Each core's "out" is a sum-shard of the output, column-rotated by
    # -i*DS. Un-rotate and sum to unshard.
    total = np.zeros((B, D), np.float32)
    for i in range(NC):
        total += np.roll(res.results[i]["out"], i * DS, axis=1)
    return total


if __name__ == "__main__":
    rng = np.random.default_rng(0)
    ins = {
        "x": rng.standard_normal((B, D), np.float32),
        "ada_emb": rng.standard_normal((B, ADA), np.float32),
        "base": rng.standard_normal((D, D), np.float32),
        "w1": rng.standard_normal((ADA, INTER), np.float32) / np.sqrt(ADA),
        "b1": rng.standard_normal((INTER,), np.float32) / np.sqrt(ADA),
        "w2": rng.standard_normal((INTER, D * RANK * 2), np.float32) / np.sqrt(INTER),
        "b2": rng.standard_normal((D * RANK * 2,), np.float32) / np.sqrt(INTER),
    }
    out = kernel(**ins)
    print("out", out.shape, out.dtype, float(np.abs(out).mean()))
